# revision 26
# baseline (speedup 1.0000x reference)
"""Trainium2 Bass kernel for nn_LinearLLM: out[b,t,v] = sum_{s>=t,w} x[b,s,w]*W[s,w,t,v] + bias.

Strategy: shard the CONTRACTION axis s across the 8 cores (cyclic over
128-row K-chunks = 2 s-values x 64 w), each core computing partial sums
for ALL 3078 = 513*6 output (t,v) columns; the 8 bf16 partials are summed
on host. This cuts LDWEIGHTS to 33 loads/core (vs 257 for t-sharding)
and gives every matmul a wide moving operand.

Weights are quantized to fp8 e3m4 (x2048, power of two) with a greedy
error-feedback rounding that near-cancels the quantization error inside
the 6-dim subspace spanned by the (also e3m4) embedding rows — measured
end-to-end rel err ~3e-3 vs 1.8e-2 for round-to-nearest. fp8 halves the
weight DMA stream (6.5MB/core), the binding resource.

Per core: 32 regular K-chunks in DESCENDING s order (position i has
uniform padded width 96*(32-i) so all cores run one SPMD program) + a
1/8 column slice of the final s=512 chunk. PSUM banks 0-5 accumulate the
3072 main columns; as s descends, high-t columns stop receiving
contributions, so banks drain (cast bf16 + DMA out) progressively while
compute continues.
"""
import numpy as np
import ml_dtypes

from concourse import bacc, tile
from concourse.bass_utils import run_bass_kernel_spmd
import concourse.mybir as mybir

B, L1, EMB, V, NCORES = 128, 513, 64, 6, 8
NPOS = 32                      # regular K-chunk positions per core
SCALE = 2048.0                 # weight scale 2^11 (exact rescale on host)
NCOLS = 3072                   # main out cols (t < 512), 6 banks x 512
XCOLS = 385                    # per-core col slice of the s=512 chunk
OUTC = NCOLS + XCOLS
W_DT = mybir.dt.float8e3
NP_W = ml_dtypes.float8_e3m4

WIDTHS = [96 * (NPOS - i) for i in range(NPOS)]          # 3072 ... 96
XT_COLS = (NPOS + 1) * 128                               # 4224


def _groups():
    gs, cur, acc = [[0]], [], 0     # tiny first group -> earliest first MM
    for i in range(1, NPOS):
        cur.append(i)
        acc += WIDTHS[i]
        if acc >= 4096:
            gs.append(cur)
            cur, acc = [], 0
    if cur:
        gs.append(cur)
    return gs


GROUPS = _groups()
SLAB_MAIN = 128 * sum(WIDTHS)
SLAB_TOTAL = SLAB_MAIN + 64 * XCOLS
# last position whose width still covers psum bank j (drain point)
I_STOP = [max(i for i in range(NPOS) if WIDTHS[i] > 512 * j) for j in range(6)]

_CACHE = {}


def _build():
    if "nc" in _CACHE:
        return _CACHE["nc"]
    nc = bacc.Bacc("TRN2", target_bir_lowering=False, debug=False,
                   num_devices=NCORES)
    xt_dram = nc.declare_dram_parameter("xt", [128, XT_COLS], W_DT,
                                        isOutput=False)
    slab_dram = nc.declare_dram_parameter("slab", [SLAB_TOTAL], W_DT,
                                          isOutput=False)
    out_dram = nc.declare_dram_parameter("out", [128, OUTC],
                                         mybir.dt.bfloat16, isOutput=True)

    def slab_ap(off, n):
        return slab_dram[off:off + 128 * n].rearrange("(p n) -> p n", p=128)

    with tile.TileContext(nc) as tc:
        with (
            tc.tile_pool(name="io", bufs=1) as iop,
            tc.tile_pool(name="ps", bufs=1, space="PSUM") as psp,
        ):
            NA = 8                     # positions in the first xt piece
            xtA = iop.tile([128, NA * 128], W_DT, tag="xtA")
            xtB = iop.tile([128, (NPOS + 1 - NA) * 128], W_DT, tag="xtB")
            wgs = [iop.tile([128, sum(WIDTHS[i] for i in g)], W_DT,
                            tag=f"g{gi}", name=f"wg{gi}")
                   for gi, g in enumerate(GROUPS)]
            w32 = iop.tile([64, XCOLS], W_DT, tag="w32")
            obufs = [iop.tile([128, 512], mybir.dt.bfloat16, tag=f"o{j}",
                              name=f"ob{j}") for j in range(6)]
            ox = iop.tile([128, XCOLS], mybir.dt.bfloat16, tag="ox")
            pss = [psp.tile([128, 512], mybir.dt.float32, tag=f"ps{j}",
                            name=f"ps{j}") for j in range(6)]
            psx = psp.tile([128, XCOLS], mybir.dt.float32, tag="psx")

            # --- queue input DMAs on TWO issue rings (SP + Activation):
            # a single HWDGE ring sustained only ~255 B/ns, below the
            # ~307 B/ns the PE consumes; two rings overlap descriptor
            # handoff gaps. Even groups + xtA on SP, odd + xtB/w32 on Act.
            off = 0
            goffs = []
            for g in GROUPS:
                goffs.append(off)
                off += 128 * sum(WIDTHS[i] for i in g)
            nc.sync.dma_start(xtA[:], xt_dram[:, :NA * 128])
            for gi in range(0, len(GROUPS), 2):
                wsum = sum(WIDTHS[i] for i in GROUPS[gi])
                nc.sync.dma_start(wgs[gi][:], slab_ap(goffs[gi], wsum))
            scalar_order = [1, 3, "xtB", "w32", 5, 7, 9]
            scalar_order += [gi for gi in range(11, len(GROUPS), 2)]
            for tok in scalar_order:
                if tok == "xtB":
                    nc.scalar.dma_start(xtB[:], xt_dram[:, NA * 128:])
                elif tok == "w32":
                    nc.scalar.dma_start(
                        w32[:],
                        slab_dram[SLAB_MAIN:SLAB_MAIN + 64 * XCOLS]
                        .rearrange("(p n) -> p n", p=64))
                elif tok < len(GROUPS):
                    wsum = sum(WIDTHS[i] for i in GROUPS[tok])
                    nc.scalar.dma_start(wgs[tok][:],
                                        slab_ap(goffs[tok], wsum))

            def lhsT(i):
                if i < NA:
                    return xtA[:, 128 * i:128 * (i + 1)]
                return xtB[:, 128 * (i - NA):128 * (i - NA + 1)]

            def drain(j, src_ps, obuf, cols, dst0):
                # DVE-only copies: no scalar ACTIVATE -> no ACT_TABLE_LOAD
                # delaying the Activation ring's input issues
                nc.vector.tensor_copy(obuf[:, :cols], src_ps[:, :cols])
                nc.sync.dma_start(out_dram[:, dst0:dst0 + cols],
                                  obuf[:, :cols])

            # --- contraction: positions descending in s ---
            for gi, g in enumerate(GROUPS):
                off_in_g = 0
                for i in g:
                    w_i = WIDTHS[i]
                    for j in range((w_i + 511) // 512):
                        c0, c1 = 512 * j, min(512 * (j + 1), w_i)
                        nc.tensor.matmul(
                            pss[j][:, :c1 - c0],
                            lhsT(i),
                            wgs[gi][:, off_in_g + c0:off_in_g + c1],
                            start=(i == 0),
                            stop=(i == I_STOP[j]),
                        )
                    off_in_g += w_i
                    if i == 8:
                        # s=512 chunk (K=64), own bank, single matmul
                        nc.tensor.matmul(psx[:], xtB[0:64, 3072:3200],
                                         w32[0:64, :], start=True, stop=True)
                        drain(-1, psx, ox, XCOLS, NCOLS)
                    for j in range(6):
                        if I_STOP[j] == i:
                            drain(j, pss[j], obufs[j], 512, 512 * j)

    nc.compile()
    _CACHE["nc"] = nc
    return nc


def _quantize_weights(emb, W):
    """Greedy error-feedback e3m4 quantization of SCALE*W.

    Returns (xq8 (6,64) e3m4, Wq8 (513,513,6,64) e3m4 scaled, masked t<=s).
    Rounding of each 64-element w-row chooses floor/ceil per element to
    cancel the running residual r = A(q-w) + b0 where A = dequantized
    e3m4 embedding and b0 compensates the embedding's own quant error.
    """
    emb = np.asarray(emb, np.float32)
    W = np.asarray(W, np.float32)
    xq8 = emb.astype(NP_W)
    xq = xq8.astype(np.float32)                     # (6,64)
    ex = xq - emb

    Ws = W * SCALE                                  # (513,64,513,6) fp32
    B0 = np.tensordot(ex, Ws, axes=([1], [1]))      # (6,513,513,6)

    Wr = np.ascontiguousarray(Ws.transpose(0, 2, 3, 1)).reshape(-1, EMB)
    del Ws
    s_idx = np.repeat(np.arange(L1), L1 * V)
    t_idx = np.tile(np.repeat(np.arange(L1), V), L1)
    valid = t_idx <= s_idx
    Wv = np.ascontiguousarray(Wr[valid])            # (Nv, 64)
    r = np.ascontiguousarray(
        B0.transpose(1, 2, 3, 0).reshape(-1, V)[valid])
    del B0

    allb = np.arange(256, dtype=np.uint8)
    vals = allb.view(NP_W).astype(np.float32)
    grid = np.unique(vals[np.isfinite(vals)])
    lo_i = np.searchsorted(grid, Wv, side="right") - 1
    lo = grid[np.clip(lo_i, 0, len(grid) - 1)]
    hi = grid[np.clip(lo_i + 1, 0, len(grid) - 1)]
    del lo_i
    Q = np.empty_like(Wv)

    A = xq.T.copy()                                 # (64, 6)
    order = np.argsort(-np.linalg.norm(A, axis=1))
    for j in order:
        aj = A[j]
        n2 = float(aj @ aj)
        g = r @ aj
        dlo = lo[:, j] - Wv[:, j]
        dhi = hi[:, j] - Wv[:, j]
        pick_hi = 2 * g * dhi + dhi * dhi * n2 < 2 * g * dlo + dlo * dlo * n2
        d = np.where(pick_hi, dhi, dlo)
        Q[:, j] = np.where(pick_hi, hi[:, j], lo[:, j])
        r += d[:, None] * aj
    for j in order:                                 # one refinement sweep
        aj = A[j]
        n2 = float(aj @ aj)
        g = r @ aj
        cur = Q[:, j]
        other = np.where(cur == lo[:, j], hi[:, j], lo[:, j])
        dd = other - cur
        flip = 2 * g * dd + dd * dd * n2 < 0
        Q[:, j] = np.where(flip, other, cur)
        r += np.where(flip, dd, 0.0)[:, None] * aj

    Wq = np.zeros_like(Wr)
    Wq[valid] = Q
    Wq8 = Wq.reshape(L1, L1, V, EMB).astype(NP_W)   # (s,t,v,w)
    return xq8, Wq8


def _prep_inputs(src, embedding, weight):
    src = np.asarray(src)
    xq8, Wq8 = _quantize_weights(embedding, weight)

    xfull = xq8[src]                                # (B, 513, 64) e3m4
    row512 = np.ascontiguousarray(
        Wq8[512].transpose(2, 0, 1)).reshape(EMB, L1 * V)   # (64, 3078)

    in_maps = []
    for c in range(NCORES):
        ks = [8 * (NPOS - 1 - i) + c for i in range(NPOS)]
        s_arr = np.array([[2 * k, 2 * k + 1] for k in ks])   # (32,2)
        sel = xfull[:, s_arr, :]                     # (B,32,2,64)
        xt = np.zeros((128, NPOS + 1, 128), NP_W)
        xt[:, :NPOS, :] = sel.transpose(2, 3, 1, 0).reshape(128, NPOS, B)
        xt[:EMB, NPOS, :] = xfull[:, 512, :].T
        xt2 = np.ascontiguousarray(xt.reshape(128, XT_COLS))

        parts = []
        for g in GROUPS:
            blks = []
            for i in g:
                k = ks[i]
                t_hi = WIDTHS[i] // V
                arr = Wq8[2 * k:2 * k + 2, :t_hi, :, :]     # (2,t_hi,6,64)
                blks.append(arr.transpose(0, 3, 1, 2).reshape(128, WIDTHS[i]))
            parts.append(np.ascontiguousarray(
                np.concatenate(blks, axis=1)).reshape(-1))
        w32 = np.zeros((64, XCOLS), NP_W)
        c0 = XCOLS * c
        c1 = min(c0 + XCOLS, L1 * V)
        w32[:, :c1 - c0] = row512[:, c0:c1]
        parts.append(w32.reshape(-1))
        slab = np.concatenate(parts)
        assert slab.shape[0] == SLAB_TOTAL
        in_maps.append({"xt": xt2, "slab": slab})
    return in_maps


def _unshard(results, bias):
    full = np.zeros((B, L1 * V), np.float32)
    for c in range(NCORES):
        o = results[c]["out"].astype(np.float32)
        full[:, :NCOLS] += o[:, :NCOLS]
        c0 = XCOLS * c
        c1 = min(c0 + XCOLS, L1 * V)
        full[:, c0:c1] += o[:, NCOLS:NCOLS + (c1 - c0)]
    full *= 1.0 / SCALE
    full = full.reshape(B, L1, V) + np.asarray(bias, np.float32)[None]
    return np.ascontiguousarray(full.transpose(0, 2, 1))


def kernel(src, embedding, weight, bias):
    nc = _build()
    in_maps = _prep_inputs(src, embedding, weight)
    res = run_bass_kernel_spmd(nc, in_maps, list(range(NCORES)))
    return _unshard(res.results, bias)
